# revision 22
# baseline (speedup 1.0000x reference)
"""Trainium2 Bass kernel for a cached-attention decode step (B=16, T=1, D=4096,
H=32, DK=128, S=2048), tensor-parallel over heads across 8 NeuronCores.

Sharding: each core owns 4 heads: column-sharded Wq/Wk/Wv (512 rows each),
the matching slices of the KV cache, and the matching 512 columns of Wo.
Each core computes, per local head h and batch b:
    q, k_new, v_new   (projections of x)
    scores = qK^T (with k_new scattered into the cache position start_pos)
    P = exp(scores)   (softmax max-subtraction skipped: scores are O(1)-scale)
    AO_unnorm = P @ V (cache rows; the new-token term added via a small
                      transposed correction matmul)
    Z = sum(P)
    y_h = AO_unnorm^T @ Wo_slice   (per-head, unnormalized)
Host divides y_h by Z per (head, batch), sums over heads and cores.
This is numerically identical to softmax attention because the Wo projection
is linear in AO.

Datatypes: K cache, weights, q/p fp16. V cache float8_e3m4 (4 mantissa bits;
values ~N(0,1) fit e3m4's range) — the PE allows mixed-dtype matmul (e3m4
stationary x fp16 moving), which halves V's DMA bytes at ~1e-2 final rel err.
"""

from contextlib import ExitStack

import ml_dtypes
import numpy as np

import concourse.bacc as bacc
import concourse.mybir as mybir
import concourse.tile as tile
from concourse.bass_utils import run_bass_kernel_spmd
from concourse.masks import make_identity

B = 16          # batch
H = 32          # total heads
D = 4096        # model dim
DK = 128        # head dim
NCORES = 8
HL = H // NCORES            # 4 local heads per core
FL = HL * DK                # 512 local features per core
KT = D // 128               # 32 contraction tiles over D
F32 = mybir.dt.float32
F16 = mybir.dt.float16
F8 = mybir.dt.float8e3      # e3m4
AF = mybir.ActivationFunctionType
ALU = mybir.AluOpType

_PROGRAM_CACHE: dict = {}
_VARIANT = "full"  # "full" | "dma_only" | "compute_only"  (perf isolation)
K8_TILES = 12  # of the NT s-tiles, how many trailing K tiles are e3m4


def build_program(S_eff: int, repeat: int = 1, G: int = 8, kv_bufs: int = 2,
                  sc_bufs: int = 2, mi_bufs: int = 2, w_bufs: int = 2,
                  p_bufs: int = 4, wq_chunk: int = 8, wo_chunk: int = 2):
    """Emit the per-core Bass/Tile program (identical across all cores).

    repeat > 1 wraps the whole body in a hardware loop — used only for
    timing (amortizes the ~60ms per-dispatch tunnel overhead).
    """
    NT = (S_eff + 127) // 128   # s-tiles incl. the partially-valid last tile
    S_pad = NT * 128
    r_new = (S_eff - 1) % 128   # row of the new token within the last s-tile
    SV = NT * DK                # V slab width per (h, b)
    NT8 = min(K8_TILES, NT - 1)  # trailing s-tiles with K stored as e3m4
    NT16 = NT - NT8             # leading s-tiles kept fp16
    S16 = NT16 * 128
    S8 = NT8 * 128
    scatter_in_k8 = NT8 > 0 and S_eff - 1 >= S16
    assert B % G == 0

    nc = bacc.Bacc("TRN2", num_devices=NCORES)
    xt = nc.declare_dram_parameter("xt", [128, KT, B], F16, isOutput=False)
    wqkv = nc.declare_dram_parameter("wqkv", [128, KT, 3 * FL], F16, isOutput=False)
    wo = nc.declare_dram_parameter("wo", [128, HL, D], F16, isOutput=False)
    k_d = nc.declare_dram_parameter("kc", [HL, B, 128, S16], F16, isOutput=False)
    k8_d = nc.declare_dram_parameter("k8", [HL, B, 128, S8], F8, isOutput=False)
    v_d = nc.declare_dram_parameter("vc", [HL, B, 128, SV], F8, isOutput=False)
    y_d = nc.declare_dram_parameter("y", [HL, B, D], F16, isOutput=True)
    z_d = nc.declare_dram_parameter("z", [1, HL * B], F32, isOutput=True)

    with tile.TileContext(nc) as tc, ExitStack() as ctx:
        singles = ctx.enter_context(tc.tile_pool(name="singles", bufs=1))
        wpool = ctx.enter_context(tc.tile_pool(name="wpool", bufs=w_bufs))
        kpool = ctx.enter_context(tc.tile_pool(name="kp", bufs=kv_bufs))
        k8pool = ctx.enter_context(tc.tile_pool(name="k8p", bufs=kv_bufs))
        vpool = ctx.enter_context(tc.tile_pool(name="vp", bufs=kv_bufs))
        ppool = ctx.enter_context(tc.tile_pool(name="ppool", bufs=p_bufs))
        vscp = ctx.enter_context(tc.tile_pool(name="vscp", bufs=2))
        wop = ctx.enter_context(tc.tile_pool(name="wop", bufs=2))
        ysbp = ctx.enter_context(tc.tile_pool(name="ysbp", bufs=2))
        pps = ctx.enter_context(tc.tile_pool(name="proj_ps", bufs=1, space="PSUM"))
        scps = ctx.enter_context(tc.tile_pool(name="sc_ps", bufs=sc_bufs, space="PSUM"))
        aops = ctx.enter_context(tc.tile_pool(name="ao_ps", bufs=1, space="PSUM"))
        mips = ctx.enter_context(tc.tile_pool(name="misc_ps", bufs=mi_bufs, space="PSUM"))

        ident = singles.tile([128, 128], F32)
        make_identity(nc, ident)
        ones_col = singles.tile([128, 1], F32)
        nc.vector.memset(ones_col, 1.0)

        q_sb = singles.tile([B, FL], F32)       # q (scaled by 1/sqrt(DK) via Wq)
        k_sb = singles.tile([B, FL], F32)       # k_new
        vn_sb = singles.tile([B, FL], F32)      # v_new
        qT_sb = singles.tile([128, HL * B], F16)   # q^T columns per (h, b)
        kTn_sb = singles.tile([128, HL * B], F16)  # k_new^T columns per (h, b)
        pl_sb = singles.tile([B, HL], F32)      # P_last = exp(q . k_new)
        zsum_sb = singles.tile([128, HL * B], F32)
        ao_sb = singles.tile([128, HL * B], F16)
        z_sb = singles.tile([1, HL * B], F32)
        nc.vector.memset(z_sb, 0.0)
        xt_sb = singles.tile([128, KT, B], F16)
        if _VARIANT == "compute_only":
            k_fix = singles.tile([128, S16], F16)
            nc.vector.memset(k_fix, 0.01)
            k8_fix = singles.tile([128, max(S8, 128)], F8)
            nc.vector.memset(k8_fix, 0.01)
            v_fix = singles.tile([128, SV], F8)
            nc.vector.memset(v_fix, 0.01)

        def body():
            dma_only = _VARIANT == "dma_only"
            nc.sync.dma_start(out=xt_sb, in_=xt[:, :, :])

            # ---- QKV projections: out[b, f] accumulated over 32 k-tiles ----
            q_ps = pps.tile([B, FL], F32, tag="qp")
            k_ps = pps.tile([B, FL], F32, tag="kp")
            v_ps = pps.tile([B, FL], F32, tag="vp")
            for kc in range(KT // wq_chunk):
                w_sb = wpool.tile([128, wq_chunk, 3 * FL], F16, tag="w")
                nc.sync.dma_start(
                    out=w_sb, in_=wqkv[:, kc * wq_chunk : (kc + 1) * wq_chunk, :]
                )
                if dma_only:
                    continue
                for j in range(wq_chunk):
                    kt = kc * wq_chunk + j
                    lhs = xt_sb[:, kt, :]
                    st, sp = kt == 0, kt == KT - 1
                    nc.tensor.matmul(
                        q_ps, lhsT=lhs, rhs=w_sb[:, j, 0:FL], start=st, stop=sp
                    )
                    nc.tensor.matmul(
                        k_ps, lhsT=lhs, rhs=w_sb[:, j, FL : 2 * FL], start=st, stop=sp
                    )
                    nc.tensor.matmul(
                        v_ps, lhsT=lhs, rhs=w_sb[:, j, 2 * FL : 3 * FL], start=st,
                        stop=sp,
                    )
            if not dma_only:
                nc.vector.tensor_copy(q_sb, q_ps)
                nc.vector.tensor_copy(k_sb, k_ps)
                nc.vector.tensor_copy(vn_sb, v_ps)

            # ---- score_last[b] = q . k_new per head; P_last = exp ----
            for h in range(HL if not dma_only else 0):
                sl_tmp = vscp.tile([B, DK], F32, tag="sl_tmp")
                sl_h = vscp.tile([B, 1], F32, tag="sl_h")
                nc.vector.tensor_mul(
                    sl_tmp,
                    q_sb[:, h * DK : (h + 1) * DK],
                    k_sb[:, h * DK : (h + 1) * DK],
                )
                nc.vector.reduce_sum(out=sl_h, in_=sl_tmp, axis=mybir.AxisListType.X)
                nc.scalar.activation(out=pl_sb[:, h : h + 1], in_=sl_h, func=AF.Exp)

            # ---- transpose q, k_new into [d, b] column layout per head ----
            for h in range(HL if not dma_only else 0):
                qt_ps = mips.tile([DK, B], F32, tag="mi")
                nc.tensor.matmul(
                    qt_ps, lhsT=q_sb[:, h * DK : (h + 1) * DK], rhs=ident[:B, :B],
                    start=True, stop=True,
                )
                nc.vector.tensor_copy(qT_sb[:, h * B : (h + 1) * B], qt_ps)
                kt_ps = mips.tile([DK, B], F32, tag="mi")
                nc.tensor.matmul(
                    kt_ps, lhsT=k_sb[:, h * DK : (h + 1) * DK], rhs=ident[:B, :B],
                    start=True, stop=True,
                )
                nc.vector.tensor_copy(kTn_sb[:, h * B : (h + 1) * B], kt_ps)

            # hoist Wo weight loads ahead of the KV stream (trims the tail)
            wo_tiles = []
            if not dma_only:
                for hc in range(HL // wo_chunk):
                    wo_sb = wop.tile([128, wo_chunk, D], F16, tag="wo")
                    nc.sync.dma_start(
                        out=wo_sb,
                        in_=wo[:, hc * wo_chunk : (hc + 1) * wo_chunk, :],
                    )
                    wo_tiles.append(wo_sb)

            # ---- attention over the cache, head by head ----
            if dma_only:
                for h in range(HL):
                    for g in range(B // G):
                        k_sb_t = kpool.tile([128, G, S16], F16, tag="kv")
                        nc.sync.dma_start(
                            out=k_sb_t,
                            in_=k_d[h, g * G : (g + 1) * G].rearrange("g p f -> p g f"),
                        )
                        if NT8 > 0:
                            k8_sb_t = k8pool.tile([128, G, S8], F8, tag="k8")
                            nc.sync.dma_start(
                                out=k8_sb_t,
                                in_=k8_d[h, g * G : (g + 1) * G].rearrange(
                                    "g p f -> p g f"
                                ),
                            )
                        v_sb_t = vpool.tile([128, G, SV], F8, tag="vv")
                        nc.sync.dma_start(
                            out=v_sb_t,
                            in_=v_d[h, g * G : (g + 1) * G].rearrange("g p f -> p g f"),
                        )
                for hc in range(HL // wo_chunk):
                    wo_sb = wop.tile([128, wo_chunk, D], F16, tag="wo")
                    nc.sync.dma_start(
                        out=wo_sb, in_=wo[:, hc * wo_chunk : (hc + 1) * wo_chunk, :]
                    )
                nc.sync.dma_start(out=z_d[:, :], in_=z_sb)
                return
            for h in range(HL):
                ao_ps = aops.tile([DK, B], F32, tag="ao")
                # correction term: AO[d, b] += P_last[b] * v_new[b, d]
                # (transposed-by-identity matmul opens the accumulation group)
                vsc = vscp.tile([B, DK], F32, tag="vsc")
                nc.vector.tensor_scalar_mul(
                    vsc,
                    in0=vn_sb[:, h * DK : (h + 1) * DK],
                    scalar1=pl_sb[:, h : h + 1],
                )
                no_pv = NT == 1 and r_new == 0  # S_eff == 1: no cache matmuls
                nc.tensor.matmul(
                    ao_ps, lhsT=vsc, rhs=ident[:B, :B], start=True, stop=no_pv
                )

                pending = []  # software-pipeline PV one bh behind scores

                def emit_pv(ent, is_last_b):
                    b_, p_sb_, v_sb_ = ent
                    n_full = NT - 1
                    for t in range(n_full):
                        last = t == n_full - 1 and r_new == 0 and is_last_b
                        nc.tensor.matmul(
                            ao_ps[:, b_ : b_ + 1],
                            lhsT=v_sb_[:, t * DK : (t + 1) * DK],
                            rhs=p_sb_[:, t : t + 1],
                            start=False,
                            stop=last,
                        )
                    if r_new > 0:
                        nc.tensor.matmul(
                            ao_ps[:, b_ : b_ + 1],
                            lhsT=v_sb_[:r_new, (NT - 1) * DK : NT * DK],
                            rhs=p_sb_[:r_new, NT - 1 : NT],
                            start=False,
                            stop=is_last_b,
                        )

                for g in range(B // G):
                    if _VARIANT == "compute_only":
                        k_grp, k8_grp, v_grp = None, None, None
                    else:
                        k_grp = kpool.tile([128, G, S16], F16, tag="kv")
                        nc.sync.dma_start(
                            out=k_grp,
                            in_=k_d[h, g * G : (g + 1) * G].rearrange("g p f -> p g f"),
                        )
                        if NT8 > 0:
                            k8_grp = k8pool.tile([128, G, S8], F8, tag="k8")
                            nc.sync.dma_start(
                                out=k8_grp,
                                in_=k8_d[h, g * G : (g + 1) * G].rearrange(
                                    "g p f -> p g f"
                                ),
                            )
                        v_grp = vpool.tile([128, G, SV], F8, tag="vv")
                        nc.sync.dma_start(
                            out=v_grp,
                            in_=v_d[h, g * G : (g + 1) * G].rearrange("g p f -> p g f"),
                        )
                    for j in range(G):
                        b = g * G + j
                        col = h * B + b
                        k_sb_b = k_fix if k_grp is None else k_grp[:, j, :]
                        k8_sb_b = (
                            (k8_fix if k8_grp is None else k8_grp[:, j, :])
                            if NT8 > 0
                            else None
                        )
                        v_sb_b = v_fix if v_grp is None else v_grp[:, j, :]
                        if _VARIANT == "full":
                            # scatter k_new into the cache column for start_pos
                            if scatter_in_k8:
                                nc.vector.tensor_copy(
                                    out=k8_sb_b[:, S_eff - 1 - S16 : S_eff - S16],
                                    in_=kTn_sb[:, col : col + 1],
                                )
                            else:
                                nc.vector.tensor_copy(
                                    out=k_sb_b[:, S_eff - 1 : S_eff],
                                    in_=kTn_sb[:, col : col + 1],
                                )
                        sc_ps = scps.tile([128, NT], F32, tag="sc")
                        for t in range(NT):
                            if t < NT16:
                                lhs_t = k_sb_b[:, t * 128 : (t + 1) * 128]
                            else:
                                tt = t - NT16
                                lhs_t = k8_sb_b[:, tt * 128 : (tt + 1) * 128]
                            nc.tensor.matmul(
                                sc_ps[:, t : t + 1],
                                lhsT=lhs_t,
                                rhs=qT_sb[:, col : col + 1],
                                start=True,
                                stop=True,
                            )
                        p_sb = ppool.tile([128, NT], F16, tag="p")
                        nc.scalar.activation(
                            out=p_sb,
                            in_=sc_ps,
                            func=AF.Exp,
                            accum_out=zsum_sb[:, col : col + 1],
                        )
                        pending.append((b, p_sb, v_sb_b))
                        if len(pending) == 2:
                            emit_pv(pending.pop(0), is_last_b=False)
                emit_pv(pending.pop(0), is_last_b=True)

                nc.vector.tensor_copy(ao_sb[:, h * B : (h + 1) * B], ao_ps)
                # Z per (h, b): sum zsum over partitions via ones-matmul
                z_ps = mips.tile([1, B], F32, tag="mi")
                nc.tensor.matmul(
                    z_ps,
                    lhsT=ones_col,
                    rhs=zsum_sb[:, h * B : (h + 1) * B],
                    start=True,
                    stop=True,
                )
                nc.vector.tensor_copy(z_sb[:, h * B : (h + 1) * B], z_ps)

            # ---- per-head output projection (unnormalized) ----
            for hc in range(HL // wo_chunk):
                wo_sb = wo_tiles[hc]
                for j in range(wo_chunk):
                    h = hc * wo_chunk + j
                    y_sb = ysbp.tile([B, D], F16, tag="ysb")
                    for oc in range(D // 512):
                        y_ps = mips.tile([B, 512], F32, tag="mi")
                        nc.tensor.matmul(
                            y_ps,
                            lhsT=ao_sb[:, h * B : (h + 1) * B],
                            rhs=wo_sb[:, j, oc * 512 : (oc + 1) * 512],
                            start=True,
                            stop=True,
                        )
                        nc.vector.tensor_copy(y_sb[:, oc * 512 : (oc + 1) * 512], y_ps)
                    nc.sync.dma_start(out=y_d[h], in_=y_sb)

            nc.sync.dma_start(out=z_d[:, :], in_=z_sb)

        if repeat == 1:
            body()
        else:
            with tc.For_i(0, repeat, 1):
                body()

    nc.compile()
    return nc


def _prep_inputs(x, k_cache, v_cache, Wq, Wk, Wv, Wo, S_eff):
    """Host-side sharding + layout prep. Returns per-core input dicts."""
    NT = (S_eff + 127) // 128
    S_pad = NT * 128
    scale = np.float32(DK ** -0.5)

    x2 = np.asarray(x, dtype=np.float32).reshape(B, D)
    xt_tiled = np.ascontiguousarray(
        x2.T.reshape(KT, 128, B).transpose(1, 0, 2).astype(np.float16)
    )  # [128, KT, B]

    k_cache = np.asarray(k_cache, dtype=np.float32)
    v_cache = np.asarray(v_cache, dtype=np.float32)

    NT8 = min(K8_TILES, NT - 1)
    S8 = NT8 * 128
    S16 = S_pad - S8
    # K^T slabs per (h,b): leading positions [0, S16) fp16, trailing as e3m4
    k_all = np.ascontiguousarray(
        k_cache[:, :, :S16, :].transpose(1, 0, 3, 2)
    ).astype(np.float16)
    k8_all = np.zeros((H, B, 128, S8), dtype=ml_dtypes.float8_e3m4)
    k8_all[:, :, :, : S_eff - S16] = (
        k_cache[:, :, S16:S_eff, :].transpose(1, 0, 3, 2)
    ).astype(ml_dtypes.float8_e3m4)
    # V-tiled slab per (h,b): [128, NT*DK] e3m4
    v_src = np.zeros((H, B, S_pad, DK), dtype=np.float32)
    v_src[:, :, :S_eff] = v_cache[:, :, :S_eff].transpose(1, 0, 2, 3)
    v_all = np.ascontiguousarray(
        v_src.reshape(H, B, NT, 128, DK)
        .transpose(0, 1, 3, 2, 4)
        .reshape(H, B, 128, NT * DK)
    ).astype(ml_dtypes.float8_e3m4)
    del v_src

    Wq = np.asarray(Wq, dtype=np.float32)
    Wk = np.asarray(Wk, dtype=np.float32)
    Wv = np.asarray(Wv, dtype=np.float32)
    Wo = np.asarray(Wo, dtype=np.float32)

    in_maps = []
    for c in range(NCORES):
        rows = slice(c * FL, (c + 1) * FL)
        wqkv_c = np.concatenate(
            [Wq[rows].T * scale, Wk[rows].T, Wv[rows].T], axis=1
        )  # (D, 3*FL)
        wqkv_tiled = np.ascontiguousarray(
            wqkv_c.reshape(KT, 128, 3 * FL).transpose(1, 0, 2).astype(np.float16)
        )
        wo_c = Wo[:, rows].T  # (FL, D)
        wo_tiled = np.ascontiguousarray(
            wo_c.reshape(HL, 128, D).transpose(1, 0, 2).astype(np.float16)
        )
        in_maps.append(
            {
                "xt": xt_tiled,
                "wqkv": wqkv_tiled,
                "wo": wo_tiled,
                "kc": np.ascontiguousarray(k_all[c * HL : (c + 1) * HL]),
                "k8": np.ascontiguousarray(k8_all[c * HL : (c + 1) * HL]),
                "vc": np.ascontiguousarray(v_all[c * HL : (c + 1) * HL]),
            }
        )
    return in_maps


def _combine(results, S_eff):
    """Host-side unshard: divide per-head partials by Z, sum everything."""
    NT = (S_eff + 127) // 128
    n_pad = NT * 128 - S_eff
    y = np.zeros((B, D), dtype=np.float64)
    for c in range(NCORES):
        z = results[c]["z"].reshape(HL, B).astype(np.float64) - n_pad
        yp = results[c]["y"].astype(np.float64)  # (HL, B, D)
        y += (yp / z[:, :, None]).sum(axis=0)
    return y.astype(np.float32).reshape(B, 1, D)


def kernel(x, k_cache, v_cache, Wq, Wk, Wv, Wo, start_pos):
    start_pos = int(np.asarray(start_pos))
    S_eff = start_pos + 1
    in_maps = _prep_inputs(x, k_cache, v_cache, Wq, Wk, Wv, Wo, S_eff)
    nc = _PROGRAM_CACHE.get(S_eff)
    if nc is None:
        nc = build_program(S_eff)
        _PROGRAM_CACHE[S_eff] = nc
    res = run_bass_kernel_spmd(nc, in_maps, core_ids=list(range(NCORES)))
    return _combine(res.results, S_eff)


# revision 23
# speedup vs baseline: 1.0189x; 1.0189x over previous
"""Trainium2 Bass kernel for a cached-attention decode step (B=16, T=1, D=4096,
H=32, DK=128, S=2048), tensor-parallel over heads across 8 NeuronCores.

Sharding: each core owns 4 heads: column-sharded Wq/Wk/Wv (512 rows each),
the matching slices of the KV cache, and the matching 512 columns of Wo.
Each core computes, per local head h and batch b:
    q, k_new, v_new   (projections of x)
    scores = qK^T (with k_new scattered into the cache position start_pos)
    P = exp(scores)   (softmax max-subtraction skipped: scores are O(1)-scale)
    AO_unnorm = P @ V (cache rows; the new-token term added via a small
                      transposed correction matmul)
    Z = sum(P)
    y_h = AO_unnorm^T @ Wo_slice   (per-head, unnormalized)
Host divides y_h by Z per (head, batch), sums over heads and cores.
This is numerically identical to softmax attention because the Wo projection
is linear in AO.

Datatypes: K cache, weights, q/p fp16. V cache float8_e3m4 (4 mantissa bits;
values ~N(0,1) fit e3m4's range) — the PE allows mixed-dtype matmul (e3m4
stationary x fp16 moving), which halves V's DMA bytes at ~1e-2 final rel err.
"""

from contextlib import ExitStack

import ml_dtypes
import numpy as np

import concourse.bacc as bacc
import concourse.mybir as mybir
import concourse.tile as tile
from concourse.bass_utils import run_bass_kernel_spmd
from concourse.masks import make_identity

B = 16          # batch
H = 32          # total heads
D = 4096        # model dim
DK = 128        # head dim
NCORES = 8
HL = H // NCORES            # 4 local heads per core
FL = HL * DK                # 512 local features per core
KT = D // 128               # 32 contraction tiles over D
F32 = mybir.dt.float32
F16 = mybir.dt.float16
F8 = mybir.dt.float8e3      # e3m4
AF = mybir.ActivationFunctionType
ALU = mybir.AluOpType

_PROGRAM_CACHE: dict = {}
_VARIANT = "full"  # "full" | "dma_only" | "compute_only"  (perf isolation)
K8_TILES = 12  # of the NT s-tiles, how many trailing K tiles are e3m4


def build_program(S_eff: int, repeat: int = 1, G: int = 4, kv_bufs: int = 3,
                  sc_bufs: int = 2, mi_bufs: int = 2, w_bufs: int = 2,
                  p_bufs: int = 4, wq_chunk: int = 8, wo_chunk: int = 2):
    """Emit the per-core Bass/Tile program (identical across all cores).

    repeat > 1 wraps the whole body in a hardware loop — used only for
    timing (amortizes the ~60ms per-dispatch tunnel overhead).
    """
    NT = (S_eff + 127) // 128   # s-tiles incl. the partially-valid last tile
    S_pad = NT * 128
    r_new = (S_eff - 1) % 128   # row of the new token within the last s-tile
    SV = NT * DK                # V slab width per (h, b)
    NT8 = min(K8_TILES, NT - 1)  # trailing s-tiles with K stored as e3m4
    NT16 = NT - NT8             # leading s-tiles kept fp16
    S16 = NT16 * 128
    S8 = NT8 * 128
    scatter_in_k8 = NT8 > 0 and S_eff - 1 >= S16
    assert B % G == 0

    nc = bacc.Bacc("TRN2", num_devices=NCORES)
    xt = nc.declare_dram_parameter("xt", [128, KT, B], F16, isOutput=False)
    wqkv = nc.declare_dram_parameter("wqkv", [128, KT, 3 * FL], F16, isOutput=False)
    wo = nc.declare_dram_parameter("wo", [128, HL, D], F16, isOutput=False)
    k_d = nc.declare_dram_parameter("kc", [HL, B, 128, S16], F16, isOutput=False)
    k8_d = nc.declare_dram_parameter("k8", [HL, B, 128, S8], F8, isOutput=False)
    v_d = nc.declare_dram_parameter("vc", [HL, B, 128, SV], F8, isOutput=False)
    y_d = nc.declare_dram_parameter("y", [HL, B, D], F16, isOutput=True)
    z_d = nc.declare_dram_parameter("z", [1, HL * B], F32, isOutput=True)

    with tile.TileContext(nc) as tc, ExitStack() as ctx:
        singles = ctx.enter_context(tc.tile_pool(name="singles", bufs=1))
        wpool = ctx.enter_context(tc.tile_pool(name="wpool", bufs=w_bufs))
        kpool = ctx.enter_context(tc.tile_pool(name="kp", bufs=kv_bufs))
        k8pool = ctx.enter_context(tc.tile_pool(name="k8p", bufs=kv_bufs))
        vpool = ctx.enter_context(tc.tile_pool(name="vp", bufs=kv_bufs))
        ppool = ctx.enter_context(tc.tile_pool(name="ppool", bufs=p_bufs))
        vscp = ctx.enter_context(tc.tile_pool(name="vscp", bufs=2))
        wop = ctx.enter_context(tc.tile_pool(name="wop", bufs=2))
        ysbp = ctx.enter_context(tc.tile_pool(name="ysbp", bufs=2))
        pps = ctx.enter_context(tc.tile_pool(name="proj_ps", bufs=1, space="PSUM"))
        scps = ctx.enter_context(tc.tile_pool(name="sc_ps", bufs=sc_bufs, space="PSUM"))
        aops = ctx.enter_context(tc.tile_pool(name="ao_ps", bufs=1, space="PSUM"))
        mips = ctx.enter_context(tc.tile_pool(name="misc_ps", bufs=mi_bufs, space="PSUM"))

        ident = singles.tile([128, 128], F32)
        make_identity(nc, ident)
        ones_col = singles.tile([128, 1], F32)
        nc.vector.memset(ones_col, 1.0)

        q_sb = singles.tile([B, FL], F32)       # q (scaled by 1/sqrt(DK) via Wq)
        k_sb = singles.tile([B, FL], F32)       # k_new
        vn_sb = singles.tile([B, FL], F32)      # v_new
        qT_sb = singles.tile([128, HL * B], F16)   # q^T columns per (h, b)
        kTn_sb = singles.tile([128, HL * B], F16)  # k_new^T columns per (h, b)
        pl_sb = singles.tile([B, HL], F32)      # P_last = exp(q . k_new)
        zsum_sb = singles.tile([128, HL * B], F32)
        ao_sb = singles.tile([128, HL * B], F16)
        z_sb = singles.tile([1, HL * B], F32)
        nc.vector.memset(z_sb, 0.0)
        xt_sb = singles.tile([128, KT, B], F16)
        if _VARIANT == "compute_only":
            k_fix = singles.tile([128, S16], F16)
            nc.vector.memset(k_fix, 0.01)
            k8_fix = singles.tile([128, max(S8, 128)], F8)
            nc.vector.memset(k8_fix, 0.01)
            v_fix = singles.tile([128, SV], F8)
            nc.vector.memset(v_fix, 0.01)

        def body():
            dma_only = _VARIANT == "dma_only"
            nc.sync.dma_start(out=xt_sb, in_=xt[:, :, :])

            # ---- QKV projections: out[b, f] accumulated over 32 k-tiles ----
            q_ps = pps.tile([B, FL], F32, tag="qp")
            k_ps = pps.tile([B, FL], F32, tag="kp")
            v_ps = pps.tile([B, FL], F32, tag="vp")
            for kc in range(KT // wq_chunk):
                w_sb = wpool.tile([128, wq_chunk, 3 * FL], F16, tag="w")
                nc.sync.dma_start(
                    out=w_sb, in_=wqkv[:, kc * wq_chunk : (kc + 1) * wq_chunk, :]
                )
                if dma_only:
                    continue
                for j in range(wq_chunk):
                    kt = kc * wq_chunk + j
                    lhs = xt_sb[:, kt, :]
                    st, sp = kt == 0, kt == KT - 1
                    nc.tensor.matmul(
                        q_ps, lhsT=lhs, rhs=w_sb[:, j, 0:FL], start=st, stop=sp
                    )
                    nc.tensor.matmul(
                        k_ps, lhsT=lhs, rhs=w_sb[:, j, FL : 2 * FL], start=st, stop=sp
                    )
                    nc.tensor.matmul(
                        v_ps, lhsT=lhs, rhs=w_sb[:, j, 2 * FL : 3 * FL], start=st,
                        stop=sp,
                    )
            if not dma_only:
                nc.vector.tensor_copy(q_sb, q_ps)
                nc.vector.tensor_copy(k_sb, k_ps)
                nc.vector.tensor_copy(vn_sb, v_ps)

            # ---- score_last[b] = q . k_new per head; P_last = exp ----
            for h in range(HL if not dma_only else 0):
                sl_tmp = vscp.tile([B, DK], F32, tag="sl_tmp")
                sl_h = vscp.tile([B, 1], F32, tag="sl_h")
                nc.vector.tensor_mul(
                    sl_tmp,
                    q_sb[:, h * DK : (h + 1) * DK],
                    k_sb[:, h * DK : (h + 1) * DK],
                )
                nc.vector.reduce_sum(out=sl_h, in_=sl_tmp, axis=mybir.AxisListType.X)
                nc.scalar.activation(out=pl_sb[:, h : h + 1], in_=sl_h, func=AF.Exp)

            # ---- transpose q, k_new into [d, b] column layout per head ----
            for h in range(HL if not dma_only else 0):
                qt_ps = mips.tile([DK, B], F32, tag="mi")
                nc.tensor.matmul(
                    qt_ps, lhsT=q_sb[:, h * DK : (h + 1) * DK], rhs=ident[:B, :B],
                    start=True, stop=True,
                )
                nc.vector.tensor_copy(qT_sb[:, h * B : (h + 1) * B], qt_ps)
                kt_ps = mips.tile([DK, B], F32, tag="mi")
                nc.tensor.matmul(
                    kt_ps, lhsT=k_sb[:, h * DK : (h + 1) * DK], rhs=ident[:B, :B],
                    start=True, stop=True,
                )
                nc.vector.tensor_copy(kTn_sb[:, h * B : (h + 1) * B], kt_ps)

            # hoist Wo weight loads ahead of the KV stream (trims the tail)
            wo_tiles = []
            if not dma_only:
                for hc in range(HL // wo_chunk):
                    wo_sb = wop.tile([128, wo_chunk, D], F16, tag="wo")
                    nc.sync.dma_start(
                        out=wo_sb,
                        in_=wo[:, hc * wo_chunk : (hc + 1) * wo_chunk, :],
                    )
                    wo_tiles.append(wo_sb)

            # ---- attention over the cache, head by head ----
            if dma_only:
                for h in range(HL):
                    for g in range(B // G):
                        k_sb_t = kpool.tile([128, G, S16], F16, tag="kv")
                        nc.sync.dma_start(
                            out=k_sb_t,
                            in_=k_d[h, g * G : (g + 1) * G].rearrange("g p f -> p g f"),
                        )
                        if NT8 > 0:
                            k8_sb_t = k8pool.tile([128, G, S8], F8, tag="k8")
                            nc.sync.dma_start(
                                out=k8_sb_t,
                                in_=k8_d[h, g * G : (g + 1) * G].rearrange(
                                    "g p f -> p g f"
                                ),
                            )
                        v_sb_t = vpool.tile([128, G, SV], F8, tag="vv")
                        nc.sync.dma_start(
                            out=v_sb_t,
                            in_=v_d[h, g * G : (g + 1) * G].rearrange("g p f -> p g f"),
                        )
                for hc in range(HL // wo_chunk):
                    wo_sb = wop.tile([128, wo_chunk, D], F16, tag="wo")
                    nc.sync.dma_start(
                        out=wo_sb, in_=wo[:, hc * wo_chunk : (hc + 1) * wo_chunk, :]
                    )
                nc.sync.dma_start(out=z_d[:, :], in_=z_sb)
                return
            for h in range(HL):
                ao_ps = aops.tile([DK, B], F32, tag="ao")
                # correction term: AO[d, b] += P_last[b] * v_new[b, d]
                # (transposed-by-identity matmul opens the accumulation group)
                vsc = vscp.tile([B, DK], F32, tag="vsc")
                nc.vector.tensor_scalar_mul(
                    vsc,
                    in0=vn_sb[:, h * DK : (h + 1) * DK],
                    scalar1=pl_sb[:, h : h + 1],
                )
                no_pv = NT == 1 and r_new == 0  # S_eff == 1: no cache matmuls
                nc.tensor.matmul(
                    ao_ps, lhsT=vsc, rhs=ident[:B, :B], start=True, stop=no_pv
                )

                pending = []  # software-pipeline PV one bh behind scores

                def emit_pv(ent, is_last_b):
                    b_, p_sb_, v_sb_ = ent
                    n_full = NT - 1
                    for t in range(n_full):
                        last = t == n_full - 1 and r_new == 0 and is_last_b
                        nc.tensor.matmul(
                            ao_ps[:, b_ : b_ + 1],
                            lhsT=v_sb_[:, t * DK : (t + 1) * DK],
                            rhs=p_sb_[:, t : t + 1],
                            start=False,
                            stop=last,
                        )
                    if r_new > 0:
                        nc.tensor.matmul(
                            ao_ps[:, b_ : b_ + 1],
                            lhsT=v_sb_[:r_new, (NT - 1) * DK : NT * DK],
                            rhs=p_sb_[:r_new, NT - 1 : NT],
                            start=False,
                            stop=is_last_b,
                        )

                for g in range(B // G):
                    if _VARIANT == "compute_only":
                        k_grp, k8_grp, v_grp = None, None, None
                    else:
                        k_grp = kpool.tile([128, G, S16], F16, tag="kv")
                        nc.sync.dma_start(
                            out=k_grp,
                            in_=k_d[h, g * G : (g + 1) * G].rearrange("g p f -> p g f"),
                        )
                        if NT8 > 0:
                            k8_grp = k8pool.tile([128, G, S8], F8, tag="k8")
                            nc.sync.dma_start(
                                out=k8_grp,
                                in_=k8_d[h, g * G : (g + 1) * G].rearrange(
                                    "g p f -> p g f"
                                ),
                            )
                        v_grp = vpool.tile([128, G, SV], F8, tag="vv")
                        nc.sync.dma_start(
                            out=v_grp,
                            in_=v_d[h, g * G : (g + 1) * G].rearrange("g p f -> p g f"),
                        )
                    for j in range(G):
                        b = g * G + j
                        col = h * B + b
                        k_sb_b = k_fix if k_grp is None else k_grp[:, j, :]
                        k8_sb_b = (
                            (k8_fix if k8_grp is None else k8_grp[:, j, :])
                            if NT8 > 0
                            else None
                        )
                        v_sb_b = v_fix if v_grp is None else v_grp[:, j, :]
                        if _VARIANT == "full":
                            # scatter k_new into the cache column for start_pos
                            if scatter_in_k8:
                                nc.vector.tensor_copy(
                                    out=k8_sb_b[:, S_eff - 1 - S16 : S_eff - S16],
                                    in_=kTn_sb[:, col : col + 1],
                                )
                            else:
                                nc.vector.tensor_copy(
                                    out=k_sb_b[:, S_eff - 1 : S_eff],
                                    in_=kTn_sb[:, col : col + 1],
                                )
                        sc_ps = scps.tile([128, NT], F32, tag="sc")
                        for t in range(NT):
                            if t < NT16:
                                lhs_t = k_sb_b[:, t * 128 : (t + 1) * 128]
                            else:
                                tt = t - NT16
                                lhs_t = k8_sb_b[:, tt * 128 : (tt + 1) * 128]
                            nc.tensor.matmul(
                                sc_ps[:, t : t + 1],
                                lhsT=lhs_t,
                                rhs=qT_sb[:, col : col + 1],
                                start=True,
                                stop=True,
                            )
                        p_sb = ppool.tile([128, NT], F16, tag="p")
                        nc.scalar.activation(
                            out=p_sb,
                            in_=sc_ps,
                            func=AF.Exp,
                            accum_out=zsum_sb[:, col : col + 1],
                        )
                        pending.append((b, p_sb, v_sb_b))
                        if len(pending) == 2:
                            emit_pv(pending.pop(0), is_last_b=False)
                emit_pv(pending.pop(0), is_last_b=True)

                nc.vector.tensor_copy(ao_sb[:, h * B : (h + 1) * B], ao_ps)
                # Z per (h, b): sum zsum over partitions via ones-matmul
                z_ps = mips.tile([1, B], F32, tag="mi")
                nc.tensor.matmul(
                    z_ps,
                    lhsT=ones_col,
                    rhs=zsum_sb[:, h * B : (h + 1) * B],
                    start=True,
                    stop=True,
                )
                nc.vector.tensor_copy(z_sb[:, h * B : (h + 1) * B], z_ps)

            # ---- per-head output projection (unnormalized) ----
            for hc in range(HL // wo_chunk):
                wo_sb = wo_tiles[hc]
                for j in range(wo_chunk):
                    h = hc * wo_chunk + j
                    y_sb = ysbp.tile([B, D], F16, tag="ysb")
                    for oc in range(D // 512):
                        y_ps = mips.tile([B, 512], F32, tag="mi")
                        nc.tensor.matmul(
                            y_ps,
                            lhsT=ao_sb[:, h * B : (h + 1) * B],
                            rhs=wo_sb[:, j, oc * 512 : (oc + 1) * 512],
                            start=True,
                            stop=True,
                        )
                        nc.vector.tensor_copy(y_sb[:, oc * 512 : (oc + 1) * 512], y_ps)
                    nc.sync.dma_start(out=y_d[h], in_=y_sb)

            nc.sync.dma_start(out=z_d[:, :], in_=z_sb)

        if repeat == 1:
            body()
        else:
            with tc.For_i(0, repeat, 1):
                body()

    nc.compile()
    return nc


def _prep_inputs(x, k_cache, v_cache, Wq, Wk, Wv, Wo, S_eff):
    """Host-side sharding + layout prep. Returns per-core input dicts."""
    NT = (S_eff + 127) // 128
    S_pad = NT * 128
    scale = np.float32(DK ** -0.5)

    x2 = np.asarray(x, dtype=np.float32).reshape(B, D)
    xt_tiled = np.ascontiguousarray(
        x2.T.reshape(KT, 128, B).transpose(1, 0, 2).astype(np.float16)
    )  # [128, KT, B]

    k_cache = np.asarray(k_cache, dtype=np.float32)
    v_cache = np.asarray(v_cache, dtype=np.float32)

    NT8 = min(K8_TILES, NT - 1)
    S8 = NT8 * 128
    S16 = S_pad - S8
    # K^T slabs per (h,b): leading positions [0, S16) fp16, trailing as e3m4
    k_all = np.ascontiguousarray(
        k_cache[:, :, :S16, :].transpose(1, 0, 3, 2)
    ).astype(np.float16)
    k8_all = np.zeros((H, B, 128, S8), dtype=ml_dtypes.float8_e3m4)
    k8_all[:, :, :, : S_eff - S16] = (
        k_cache[:, :, S16:S_eff, :].transpose(1, 0, 3, 2)
    ).astype(ml_dtypes.float8_e3m4)
    # V-tiled slab per (h,b): [128, NT*DK] e3m4
    v_src = np.zeros((H, B, S_pad, DK), dtype=np.float32)
    v_src[:, :, :S_eff] = v_cache[:, :, :S_eff].transpose(1, 0, 2, 3)
    v_all = np.ascontiguousarray(
        v_src.reshape(H, B, NT, 128, DK)
        .transpose(0, 1, 3, 2, 4)
        .reshape(H, B, 128, NT * DK)
    ).astype(ml_dtypes.float8_e3m4)
    del v_src

    Wq = np.asarray(Wq, dtype=np.float32)
    Wk = np.asarray(Wk, dtype=np.float32)
    Wv = np.asarray(Wv, dtype=np.float32)
    Wo = np.asarray(Wo, dtype=np.float32)

    in_maps = []
    for c in range(NCORES):
        rows = slice(c * FL, (c + 1) * FL)
        wqkv_c = np.concatenate(
            [Wq[rows].T * scale, Wk[rows].T, Wv[rows].T], axis=1
        )  # (D, 3*FL)
        wqkv_tiled = np.ascontiguousarray(
            wqkv_c.reshape(KT, 128, 3 * FL).transpose(1, 0, 2).astype(np.float16)
        )
        wo_c = Wo[:, rows].T  # (FL, D)
        wo_tiled = np.ascontiguousarray(
            wo_c.reshape(HL, 128, D).transpose(1, 0, 2).astype(np.float16)
        )
        in_maps.append(
            {
                "xt": xt_tiled,
                "wqkv": wqkv_tiled,
                "wo": wo_tiled,
                "kc": np.ascontiguousarray(k_all[c * HL : (c + 1) * HL]),
                "k8": np.ascontiguousarray(k8_all[c * HL : (c + 1) * HL]),
                "vc": np.ascontiguousarray(v_all[c * HL : (c + 1) * HL]),
            }
        )
    return in_maps


def _combine(results, S_eff):
    """Host-side unshard: divide per-head partials by Z, sum everything."""
    NT = (S_eff + 127) // 128
    n_pad = NT * 128 - S_eff
    y = np.zeros((B, D), dtype=np.float64)
    for c in range(NCORES):
        z = results[c]["z"].reshape(HL, B).astype(np.float64) - n_pad
        yp = results[c]["y"].astype(np.float64)  # (HL, B, D)
        y += (yp / z[:, :, None]).sum(axis=0)
    return y.astype(np.float32).reshape(B, 1, D)


def kernel(x, k_cache, v_cache, Wq, Wk, Wv, Wo, start_pos):
    start_pos = int(np.asarray(start_pos))
    S_eff = start_pos + 1
    in_maps = _prep_inputs(x, k_cache, v_cache, Wq, Wk, Wv, Wo, S_eff)
    nc = _PROGRAM_CACHE.get(S_eff)
    if nc is None:
        nc = build_program(S_eff)
        _PROGRAM_CACHE[S_eff] = nc
    res = run_bass_kernel_spmd(nc, in_maps, core_ids=list(range(NCORES)))
    return _combine(res.results, S_eff)


# revision 25
# speedup vs baseline: 1.0484x; 1.0289x over previous
"""Trainium2 Bass kernel for a cached-attention decode step (B=16, T=1, D=4096,
H=32, DK=128, S=2048), tensor-parallel over heads across 8 NeuronCores.

Sharding: each core owns 4 heads: column-sharded Wq/Wk/Wv (512 rows each),
the matching slices of the KV cache, and the matching 512 columns of Wo.
Each core computes, per local head h and batch b:
    q, k_new, v_new   (projections of x)
    scores = qK^T (with k_new scattered into the cache position start_pos)
    P = exp(scores)   (softmax max-subtraction skipped: scores are O(1)-scale)
    AO_unnorm = P @ V (cache rows; the new-token term added via a small
                      transposed correction matmul)
    Z = sum(P)
    y_h = AO_unnorm^T @ Wo_slice   (per-head, unnormalized)
Host divides y_h by Z per (head, batch), sums over heads and cores.
This is numerically identical to softmax attention because the Wo projection
is linear in AO.

Datatypes: K cache, weights, q/p fp16. V cache float8_e3m4 (4 mantissa bits;
values ~N(0,1) fit e3m4's range) — the PE allows mixed-dtype matmul (e3m4
stationary x fp16 moving), which halves V's DMA bytes at ~1e-2 final rel err.
"""

from contextlib import ExitStack

import ml_dtypes
import numpy as np

import concourse.bacc as bacc
import concourse.mybir as mybir
import concourse.tile as tile
from concourse.bass_utils import run_bass_kernel_spmd
from concourse.masks import make_identity

B = 16          # batch
H = 32          # total heads
D = 4096        # model dim
DK = 128        # head dim
NCORES = 8
HL = H // NCORES            # 4 local heads per core
FL = HL * DK                # 512 local features per core
KT = D // 128               # 32 contraction tiles over D
F32 = mybir.dt.float32
F16 = mybir.dt.float16
F8 = mybir.dt.float8e3      # e3m4
AF = mybir.ActivationFunctionType
ALU = mybir.AluOpType

_PROGRAM_CACHE: dict = {}
_VARIANT = "full"  # "full" | "dma_only" | "compute_only"  (perf isolation)
K8_TILES = 12  # of the NT s-tiles, how many trailing K tiles are e3m4


def build_program(S_eff: int, repeat: int = 1, G: int = 4, kv_bufs: int = 4,
                  sc_bufs: int = 2, mi_bufs: int = 2, w_bufs: int = 2,
                  p_bufs: int = 4, wq_chunk: int = 8, wo_chunk: int = 2):
    """Emit the per-core Bass/Tile program (identical across all cores).

    repeat > 1 wraps the whole body in a hardware loop — used only for
    timing (amortizes the ~60ms per-dispatch tunnel overhead).
    """
    NT = (S_eff + 127) // 128   # s-tiles incl. the partially-valid last tile
    S_pad = NT * 128
    r_new = (S_eff - 1) % 128   # row of the new token within the last s-tile
    SV = NT * DK                # V slab width per (h, b)
    NT8 = min(K8_TILES, NT - 1)  # trailing s-tiles with K stored as e3m4
    NT16 = NT - NT8             # leading s-tiles kept fp16
    S16 = NT16 * 128
    S8 = NT8 * 128
    scatter_in_k8 = NT8 > 0 and S_eff - 1 >= S16
    assert B % G == 0

    nc = bacc.Bacc("TRN2", num_devices=NCORES)
    xt = nc.declare_dram_parameter("xt", [128, KT, B], F16, isOutput=False)
    wqkv = nc.declare_dram_parameter("wqkv", [128, KT, 2 * FL], F16, isOutput=False)
    wv8 = nc.declare_dram_parameter("wv8", [128, KT, FL], F8, isOutput=False)
    wo = nc.declare_dram_parameter("wo", [128, HL, D], F16, isOutput=False)
    k_d = nc.declare_dram_parameter("kc", [HL, B, 128, S16], F16, isOutput=False)
    k8_d = nc.declare_dram_parameter("k8", [HL, B, 128, S8], F8, isOutput=False)
    v_d = nc.declare_dram_parameter("vc", [HL, B, 128, SV], F8, isOutput=False)
    y_d = nc.declare_dram_parameter("y", [HL, B, D], F16, isOutput=True)
    z_d = nc.declare_dram_parameter("z", [1, HL * B], F32, isOutput=True)

    with tile.TileContext(nc) as tc, ExitStack() as ctx:
        singles = ctx.enter_context(tc.tile_pool(name="singles", bufs=1))
        wpool = ctx.enter_context(tc.tile_pool(name="wpool", bufs=w_bufs))
        w8pool = ctx.enter_context(tc.tile_pool(name="w8p", bufs=w_bufs))
        kpool = ctx.enter_context(tc.tile_pool(name="kp", bufs=kv_bufs))
        k8pool = ctx.enter_context(tc.tile_pool(name="k8p", bufs=kv_bufs))
        vpool = ctx.enter_context(tc.tile_pool(name="vp", bufs=kv_bufs))
        ppool = ctx.enter_context(tc.tile_pool(name="ppool", bufs=p_bufs))
        vscp = ctx.enter_context(tc.tile_pool(name="vscp", bufs=2))
        wop = ctx.enter_context(tc.tile_pool(name="wop", bufs=2))
        ysbp = ctx.enter_context(tc.tile_pool(name="ysbp", bufs=2))
        pps = ctx.enter_context(tc.tile_pool(name="proj_ps", bufs=1, space="PSUM"))
        scps = ctx.enter_context(tc.tile_pool(name="sc_ps", bufs=sc_bufs, space="PSUM"))
        aops = ctx.enter_context(tc.tile_pool(name="ao_ps", bufs=1, space="PSUM"))
        mips = ctx.enter_context(tc.tile_pool(name="misc_ps", bufs=mi_bufs, space="PSUM"))

        ident = singles.tile([128, 128], F32)
        make_identity(nc, ident)
        ones_col = singles.tile([128, 1], F32)
        nc.vector.memset(ones_col, 1.0)

        q_sb = singles.tile([B, FL], F32)       # q (scaled by 1/sqrt(DK) via Wq)
        k_sb = singles.tile([B, FL], F32)       # k_new
        vn_sb = singles.tile([B, FL], F32)      # v_new
        qT_sb = singles.tile([128, HL * B], F16)   # q^T columns per (h, b)
        kTn_sb = singles.tile([128, HL * B], F16)  # k_new^T columns per (h, b)
        pl_sb = singles.tile([B, HL], F32)      # P_last = exp(q . k_new)
        nln32_sb = singles.tile([B, 1], F32)    # -ln(32): Wv is stored x32
        nc.vector.memset(nln32_sb, -3.4657359027997265)
        zsum_sb = singles.tile([128, HL * B], F32)
        ao_sb = singles.tile([128, HL * B], F16)
        z_sb = singles.tile([1, HL * B], F32)
        nc.vector.memset(z_sb, 0.0)
        xt_sb = singles.tile([128, KT, B], F16)
        if _VARIANT == "compute_only":
            k_fix = singles.tile([128, S16], F16)
            nc.vector.memset(k_fix, 0.01)
            k8_fix = singles.tile([128, max(S8, 128)], F8)
            nc.vector.memset(k8_fix, 0.01)
            v_fix = singles.tile([128, SV], F8)
            nc.vector.memset(v_fix, 0.01)

        def body():
            dma_only = _VARIANT == "dma_only"
            nc.sync.dma_start(out=xt_sb, in_=xt[:, :, :])

            # ---- QKV projections: out[b, f] accumulated over 32 k-tiles ----
            q_ps = pps.tile([B, FL], F32, tag="qp")
            k_ps = pps.tile([B, FL], F32, tag="kp")
            v_ps = pps.tile([B, FL], F32, tag="vp")
            for kc in range(KT // wq_chunk):
                w_sb = wpool.tile([128, wq_chunk, 2 * FL], F16, tag="w")
                nc.sync.dma_start(
                    out=w_sb, in_=wqkv[:, kc * wq_chunk : (kc + 1) * wq_chunk, :]
                )
                wv_sb = w8pool.tile([128, wq_chunk, FL], F8, tag="wv")
                nc.sync.dma_start(
                    out=wv_sb, in_=wv8[:, kc * wq_chunk : (kc + 1) * wq_chunk, :]
                )
                if dma_only:
                    continue
                for j in range(wq_chunk):
                    kt = kc * wq_chunk + j
                    lhs = xt_sb[:, kt, :]
                    st, sp = kt == 0, kt == KT - 1
                    nc.tensor.matmul(
                        q_ps, lhsT=lhs, rhs=w_sb[:, j, 0:FL], start=st, stop=sp
                    )
                    nc.tensor.matmul(
                        k_ps, lhsT=lhs, rhs=w_sb[:, j, FL : 2 * FL], start=st, stop=sp
                    )
                    nc.tensor.matmul(
                        v_ps, lhsT=lhs, rhs=wv_sb[:, j, :], start=st, stop=sp
                    )
            if not dma_only:
                nc.vector.tensor_copy(q_sb, q_ps)
                nc.vector.tensor_copy(k_sb, k_ps)
                nc.vector.tensor_copy(vn_sb, v_ps)

            # ---- score_last[b] = q . k_new per head; P_last = exp ----
            for h in range(HL if not dma_only else 0):
                sl_tmp = vscp.tile([B, DK], F32, tag="sl_tmp")
                sl_h = vscp.tile([B, 1], F32, tag="sl_h")
                nc.vector.tensor_mul(
                    sl_tmp,
                    q_sb[:, h * DK : (h + 1) * DK],
                    k_sb[:, h * DK : (h + 1) * DK],
                )
                nc.vector.reduce_sum(out=sl_h, in_=sl_tmp, axis=mybir.AxisListType.X)
                nc.scalar.activation(
                    out=pl_sb[:, h : h + 1], in_=sl_h, func=AF.Exp,
                    bias=nln32_sb,
                )

            # ---- transpose q, k_new into [d, b] column layout per head ----
            for h in range(HL if not dma_only else 0):
                qt_ps = mips.tile([DK, B], F32, tag="mi")
                nc.tensor.matmul(
                    qt_ps, lhsT=q_sb[:, h * DK : (h + 1) * DK], rhs=ident[:B, :B],
                    start=True, stop=True,
                )
                nc.vector.tensor_copy(qT_sb[:, h * B : (h + 1) * B], qt_ps)
                kt_ps = mips.tile([DK, B], F32, tag="mi")
                nc.tensor.matmul(
                    kt_ps, lhsT=k_sb[:, h * DK : (h + 1) * DK], rhs=ident[:B, :B],
                    start=True, stop=True,
                )
                nc.vector.tensor_copy(kTn_sb[:, h * B : (h + 1) * B], kt_ps)

            # hoist Wo weight loads ahead of the KV stream (trims the tail)
            wo_tiles = []
            if not dma_only:
                for hc in range(HL // wo_chunk):
                    wo_sb = wop.tile([128, wo_chunk, D], F16, tag="wo")
                    nc.sync.dma_start(
                        out=wo_sb,
                        in_=wo[:, hc * wo_chunk : (hc + 1) * wo_chunk, :],
                    )
                    wo_tiles.append(wo_sb)

            # ---- attention over the cache, head by head ----
            if dma_only:
                for h in range(HL):
                    for g in range(B // G):
                        k_sb_t = kpool.tile([128, G, S16], F16, tag="kv")
                        nc.sync.dma_start(
                            out=k_sb_t,
                            in_=k_d[h, g * G : (g + 1) * G].rearrange("g p f -> p g f"),
                        )
                        if NT8 > 0:
                            k8_sb_t = k8pool.tile([128, G, S8], F8, tag="k8")
                            nc.sync.dma_start(
                                out=k8_sb_t,
                                in_=k8_d[h, g * G : (g + 1) * G].rearrange(
                                    "g p f -> p g f"
                                ),
                            )
                        v_sb_t = vpool.tile([128, G, SV], F8, tag="vv")
                        nc.sync.dma_start(
                            out=v_sb_t,
                            in_=v_d[h, g * G : (g + 1) * G].rearrange("g p f -> p g f"),
                        )
                for hc in range(HL // wo_chunk):
                    wo_sb = wop.tile([128, wo_chunk, D], F16, tag="wo")
                    nc.sync.dma_start(
                        out=wo_sb, in_=wo[:, hc * wo_chunk : (hc + 1) * wo_chunk, :]
                    )
                nc.sync.dma_start(out=z_d[:, :], in_=z_sb)
                return
            for h in range(HL):
                ao_ps = aops.tile([DK, B], F32, tag="ao")
                # correction term: AO[d, b] += P_last[b] * v_new[b, d]
                # (transposed-by-identity matmul opens the accumulation group)
                vsc = vscp.tile([B, DK], F32, tag="vsc")
                nc.vector.tensor_scalar_mul(
                    vsc,
                    in0=vn_sb[:, h * DK : (h + 1) * DK],
                    scalar1=pl_sb[:, h : h + 1],
                )
                no_pv = NT == 1 and r_new == 0  # S_eff == 1: no cache matmuls
                nc.tensor.matmul(
                    ao_ps, lhsT=vsc, rhs=ident[:B, :B], start=True, stop=no_pv
                )

                pending = []  # software-pipeline PV one bh behind scores

                def emit_pv(ent, is_last_b):
                    b_, p_sb_, v_sb_ = ent
                    n_full = NT - 1
                    for t in range(n_full):
                        last = t == n_full - 1 and r_new == 0 and is_last_b
                        nc.tensor.matmul(
                            ao_ps[:, b_ : b_ + 1],
                            lhsT=v_sb_[:, t * DK : (t + 1) * DK],
                            rhs=p_sb_[:, t : t + 1],
                            start=False,
                            stop=last,
                        )
                    if r_new > 0:
                        nc.tensor.matmul(
                            ao_ps[:, b_ : b_ + 1],
                            lhsT=v_sb_[:r_new, (NT - 1) * DK : NT * DK],
                            rhs=p_sb_[:r_new, NT - 1 : NT],
                            start=False,
                            stop=is_last_b,
                        )

                for g in range(B // G):
                    if _VARIANT == "compute_only":
                        k_grp, k8_grp, v_grp = None, None, None
                    else:
                        k_grp = kpool.tile([128, G, S16], F16, tag="kv")
                        nc.sync.dma_start(
                            out=k_grp,
                            in_=k_d[h, g * G : (g + 1) * G].rearrange("g p f -> p g f"),
                        )
                        if NT8 > 0:
                            k8_grp = k8pool.tile([128, G, S8], F8, tag="k8")
                            nc.sync.dma_start(
                                out=k8_grp,
                                in_=k8_d[h, g * G : (g + 1) * G].rearrange(
                                    "g p f -> p g f"
                                ),
                            )
                        v_grp = vpool.tile([128, G, SV], F8, tag="vv")
                        nc.sync.dma_start(
                            out=v_grp,
                            in_=v_d[h, g * G : (g + 1) * G].rearrange("g p f -> p g f"),
                        )
                    for j in range(G):
                        b = g * G + j
                        col = h * B + b
                        k_sb_b = k_fix if k_grp is None else k_grp[:, j, :]
                        k8_sb_b = (
                            (k8_fix if k8_grp is None else k8_grp[:, j, :])
                            if NT8 > 0
                            else None
                        )
                        v_sb_b = v_fix if v_grp is None else v_grp[:, j, :]
                        if _VARIANT == "full":
                            # scatter k_new into the cache column for start_pos
                            if scatter_in_k8:
                                nc.vector.tensor_copy(
                                    out=k8_sb_b[:, S_eff - 1 - S16 : S_eff - S16],
                                    in_=kTn_sb[:, col : col + 1],
                                )
                            else:
                                nc.vector.tensor_copy(
                                    out=k_sb_b[:, S_eff - 1 : S_eff],
                                    in_=kTn_sb[:, col : col + 1],
                                )
                        sc_ps = scps.tile([128, NT], F32, tag="sc")
                        for t in range(NT):
                            if t < NT16:
                                lhs_t = k_sb_b[:, t * 128 : (t + 1) * 128]
                            else:
                                tt = t - NT16
                                lhs_t = k8_sb_b[:, tt * 128 : (tt + 1) * 128]
                            nc.tensor.matmul(
                                sc_ps[:, t : t + 1],
                                lhsT=lhs_t,
                                rhs=qT_sb[:, col : col + 1],
                                start=True,
                                stop=True,
                            )
                        p_sb = ppool.tile([128, NT], F16, tag="p")
                        nc.scalar.activation(
                            out=p_sb,
                            in_=sc_ps,
                            func=AF.Exp,
                            accum_out=zsum_sb[:, col : col + 1],
                        )
                        pending.append((b, p_sb, v_sb_b))
                        if len(pending) == 2:
                            emit_pv(pending.pop(0), is_last_b=False)
                emit_pv(pending.pop(0), is_last_b=True)

                nc.vector.tensor_copy(ao_sb[:, h * B : (h + 1) * B], ao_ps)
                # Z per (h, b): sum zsum over partitions via ones-matmul
                z_ps = mips.tile([1, B], F32, tag="mi")
                nc.tensor.matmul(
                    z_ps,
                    lhsT=ones_col,
                    rhs=zsum_sb[:, h * B : (h + 1) * B],
                    start=True,
                    stop=True,
                )
                nc.vector.tensor_copy(z_sb[:, h * B : (h + 1) * B], z_ps)

            # ---- per-head output projection (unnormalized) ----
            for hc in range(HL // wo_chunk):
                wo_sb = wo_tiles[hc]
                for j in range(wo_chunk):
                    h = hc * wo_chunk + j
                    y_sb = ysbp.tile([B, D], F16, tag="ysb")
                    for oc in range(D // 512):
                        y_ps = mips.tile([B, 512], F32, tag="mi")
                        nc.tensor.matmul(
                            y_ps,
                            lhsT=ao_sb[:, h * B : (h + 1) * B],
                            rhs=wo_sb[:, j, oc * 512 : (oc + 1) * 512],
                            start=True,
                            stop=True,
                        )
                        nc.vector.tensor_copy(y_sb[:, oc * 512 : (oc + 1) * 512], y_ps)
                    nc.sync.dma_start(out=y_d[h], in_=y_sb)

            nc.sync.dma_start(out=z_d[:, :], in_=z_sb)

        if repeat == 1:
            body()
        else:
            with tc.For_i(0, repeat, 1):
                body()

    nc.compile()
    return nc


def _prep_inputs(x, k_cache, v_cache, Wq, Wk, Wv, Wo, S_eff):
    """Host-side sharding + layout prep. Returns per-core input dicts."""
    NT = (S_eff + 127) // 128
    S_pad = NT * 128
    scale = np.float32(DK ** -0.5)

    x2 = np.asarray(x, dtype=np.float32).reshape(B, D)
    xt_tiled = np.ascontiguousarray(
        x2.T.reshape(KT, 128, B).transpose(1, 0, 2).astype(np.float16)
    )  # [128, KT, B]

    k_cache = np.asarray(k_cache, dtype=np.float32)
    v_cache = np.asarray(v_cache, dtype=np.float32)

    NT8 = min(K8_TILES, NT - 1)
    S8 = NT8 * 128
    S16 = S_pad - S8
    # K^T slabs per (h,b): leading positions [0, S16) fp16, trailing as e3m4
    k_all = np.ascontiguousarray(
        k_cache[:, :, :S16, :].transpose(1, 0, 3, 2)
    ).astype(np.float16)
    k8_all = np.zeros((H, B, 128, S8), dtype=ml_dtypes.float8_e3m4)
    k8_all[:, :, :, : S_eff - S16] = (
        k_cache[:, :, S16:S_eff, :].transpose(1, 0, 3, 2)
    ).astype(ml_dtypes.float8_e3m4)
    # V-tiled slab per (h,b): [128, NT*DK] e3m4
    v_src = np.zeros((H, B, S_pad, DK), dtype=np.float32)
    v_src[:, :, :S_eff] = v_cache[:, :, :S_eff].transpose(1, 0, 2, 3)
    v_all = np.ascontiguousarray(
        v_src.reshape(H, B, NT, 128, DK)
        .transpose(0, 1, 3, 2, 4)
        .reshape(H, B, 128, NT * DK)
    ).astype(ml_dtypes.float8_e3m4)
    del v_src

    Wq = np.asarray(Wq, dtype=np.float32)
    Wk = np.asarray(Wk, dtype=np.float32)
    Wv = np.asarray(Wv, dtype=np.float32)
    Wo = np.asarray(Wo, dtype=np.float32)

    in_maps = []
    for c in range(NCORES):
        rows = slice(c * FL, (c + 1) * FL)
        wqkv_c = np.concatenate(
            [Wq[rows].T * scale, Wk[rows].T], axis=1
        )  # (D, 2*FL)
        wqkv_tiled = np.ascontiguousarray(
            wqkv_c.reshape(KT, 128, 2 * FL).transpose(1, 0, 2).astype(np.float16)
        )
        wv8_tiled = np.ascontiguousarray(
            (Wv[rows].T * np.float32(32.0))
            .reshape(KT, 128, FL).transpose(1, 0, 2)
        ).astype(ml_dtypes.float8_e3m4)
        wo_c = Wo[:, rows].T  # (FL, D)
        wo_tiled = np.ascontiguousarray(
            wo_c.reshape(HL, 128, D).transpose(1, 0, 2).astype(np.float16)
        )
        in_maps.append(
            {
                "xt": xt_tiled,
                "wqkv": wqkv_tiled,
                "wv8": wv8_tiled,
                "wo": wo_tiled,
                "kc": np.ascontiguousarray(k_all[c * HL : (c + 1) * HL]),
                "k8": np.ascontiguousarray(k8_all[c * HL : (c + 1) * HL]),
                "vc": np.ascontiguousarray(v_all[c * HL : (c + 1) * HL]),
            }
        )
    return in_maps


def _combine(results, S_eff):
    """Host-side unshard: divide per-head partials by Z, sum everything."""
    NT = (S_eff + 127) // 128
    n_pad = NT * 128 - S_eff
    y = np.zeros((B, D), dtype=np.float64)
    for c in range(NCORES):
        z = results[c]["z"].reshape(HL, B).astype(np.float64) - n_pad
        yp = results[c]["y"].astype(np.float64)  # (HL, B, D)
        y += (yp / z[:, :, None]).sum(axis=0)
    return y.astype(np.float32).reshape(B, 1, D)


def kernel(x, k_cache, v_cache, Wq, Wk, Wv, Wo, start_pos):
    start_pos = int(np.asarray(start_pos))
    S_eff = start_pos + 1
    in_maps = _prep_inputs(x, k_cache, v_cache, Wq, Wk, Wv, Wo, S_eff)
    nc = _PROGRAM_CACHE.get(S_eff)
    if nc is None:
        nc = build_program(S_eff)
        _PROGRAM_CACHE[S_eff] = nc
    res = run_bass_kernel_spmd(nc, in_maps, core_ids=list(range(NCORES)))
    return _combine(res.results, S_eff)


# revision 26
# speedup vs baseline: 1.0552x; 1.0065x over previous
"""Trainium2 Bass kernel for a cached-attention decode step (B=16, T=1, D=4096,
H=32, DK=128, S=2048), tensor-parallel over heads across 8 NeuronCores.

Sharding: each core owns 4 heads: column-sharded Wq/Wk/Wv (512 rows each),
the matching slices of the KV cache, and the matching 512 columns of Wo.
Each core computes, per local head h and batch b:
    q, k_new, v_new   (projections of x)
    scores = qK^T (with k_new scattered into the cache position start_pos)
    P = exp(scores)   (softmax max-subtraction skipped: scores are O(1)-scale)
    AO_unnorm = P @ V (cache rows; the new-token term added via a small
                      transposed correction matmul)
    Z = sum(P)
    y_h = AO_unnorm^T @ Wo_slice   (per-head, unnormalized)
Host divides y_h by Z per (head, batch), sums over heads and cores.
This is numerically identical to softmax attention because the Wo projection
is linear in AO.

Datatypes: K cache, weights, q/p fp16. V cache float8_e3m4 (4 mantissa bits;
values ~N(0,1) fit e3m4's range) — the PE allows mixed-dtype matmul (e3m4
stationary x fp16 moving), which halves V's DMA bytes at ~1e-2 final rel err.
"""

from contextlib import ExitStack

import ml_dtypes
import numpy as np

import concourse.bacc as bacc
import concourse.mybir as mybir
import concourse.tile as tile
from concourse.bass_utils import run_bass_kernel_spmd
from concourse.masks import make_identity

B = 16          # batch
H = 32          # total heads
D = 4096        # model dim
DK = 128        # head dim
NCORES = 8
HL = H // NCORES            # 4 local heads per core
FL = HL * DK                # 512 local features per core
KT = D // 128               # 32 contraction tiles over D
F32 = mybir.dt.float32
F16 = mybir.dt.float16
F8 = mybir.dt.float8e3      # e3m4
AF = mybir.ActivationFunctionType
ALU = mybir.AluOpType

_PROGRAM_CACHE: dict = {}
_VARIANT = "full"  # "full" | "dma_only" | "compute_only"  (perf isolation)
K8_TILES = 13  # of the NT s-tiles, how many trailing K tiles are e3m4


def build_program(S_eff: int, repeat: int = 1, G: int = 4, kv_bufs: int = 4,
                  sc_bufs: int = 2, mi_bufs: int = 2, w_bufs: int = 2,
                  p_bufs: int = 4, wq_chunk: int = 8, wo_chunk: int = 2):
    """Emit the per-core Bass/Tile program (identical across all cores).

    repeat > 1 wraps the whole body in a hardware loop — used only for
    timing (amortizes the ~60ms per-dispatch tunnel overhead).
    """
    NT = (S_eff + 127) // 128   # s-tiles incl. the partially-valid last tile
    S_pad = NT * 128
    r_new = (S_eff - 1) % 128   # row of the new token within the last s-tile
    SV = NT * DK                # V slab width per (h, b)
    NT8 = min(K8_TILES, NT - 1)  # trailing s-tiles with K stored as e3m4
    NT16 = NT - NT8             # leading s-tiles kept fp16
    S16 = NT16 * 128
    S8 = NT8 * 128
    scatter_in_k8 = NT8 > 0 and S_eff - 1 >= S16
    assert B % G == 0

    nc = bacc.Bacc("TRN2", num_devices=NCORES)
    xt = nc.declare_dram_parameter("xt", [128, KT, B], F16, isOutput=False)
    wqkv = nc.declare_dram_parameter("wqkv", [128, KT, 2 * FL], F16, isOutput=False)
    wv8 = nc.declare_dram_parameter("wv8", [128, KT, FL], F8, isOutput=False)
    wo = nc.declare_dram_parameter("wo", [128, HL, D], F16, isOutput=False)
    k_d = nc.declare_dram_parameter("kc", [HL, B, 128, S16], F16, isOutput=False)
    k8_d = nc.declare_dram_parameter("k8", [HL, B, 128, S8], F8, isOutput=False)
    v_d = nc.declare_dram_parameter("vc", [HL, B, 128, SV], F8, isOutput=False)
    y_d = nc.declare_dram_parameter("y", [HL, B, D], F16, isOutput=True)
    z_d = nc.declare_dram_parameter("z", [1, HL * B], F32, isOutput=True)

    with tile.TileContext(nc) as tc, ExitStack() as ctx:
        singles = ctx.enter_context(tc.tile_pool(name="singles", bufs=1))
        wpool = ctx.enter_context(tc.tile_pool(name="wpool", bufs=w_bufs))
        w8pool = ctx.enter_context(tc.tile_pool(name="w8p", bufs=w_bufs))
        kpool = ctx.enter_context(tc.tile_pool(name="kp", bufs=kv_bufs))
        k8pool = ctx.enter_context(tc.tile_pool(name="k8p", bufs=kv_bufs))
        vpool = ctx.enter_context(tc.tile_pool(name="vp", bufs=kv_bufs))
        ppool = ctx.enter_context(tc.tile_pool(name="ppool", bufs=p_bufs))
        vscp = ctx.enter_context(tc.tile_pool(name="vscp", bufs=2))
        wop = ctx.enter_context(tc.tile_pool(name="wop", bufs=2))
        ysbp = ctx.enter_context(tc.tile_pool(name="ysbp", bufs=2))
        pps = ctx.enter_context(tc.tile_pool(name="proj_ps", bufs=1, space="PSUM"))
        scps = ctx.enter_context(tc.tile_pool(name="sc_ps", bufs=sc_bufs, space="PSUM"))
        aops = ctx.enter_context(tc.tile_pool(name="ao_ps", bufs=1, space="PSUM"))
        mips = ctx.enter_context(tc.tile_pool(name="misc_ps", bufs=mi_bufs, space="PSUM"))

        ident = singles.tile([128, 128], F32)
        make_identity(nc, ident)
        ones_col = singles.tile([128, 1], F32)
        nc.vector.memset(ones_col, 1.0)

        q_sb = singles.tile([B, FL], F32)       # q (scaled by 1/sqrt(DK) via Wq)
        k_sb = singles.tile([B, FL], F32)       # k_new
        vn_sb = singles.tile([B, FL], F32)      # v_new
        qT_sb = singles.tile([128, HL * B], F16)   # q^T columns per (h, b)
        kTn_sb = singles.tile([128, HL * B], F16)  # k_new^T columns per (h, b)
        pl_sb = singles.tile([B, HL], F32)      # P_last = exp(q . k_new)
        nln32_sb = singles.tile([B, 1], F32)    # -ln(32): Wv is stored x32
        nc.vector.memset(nln32_sb, -3.4657359027997265)
        zsum_sb = singles.tile([128, HL * B], F32)
        ao_sb = singles.tile([128, HL * B], F16)
        z_sb = singles.tile([1, HL * B], F32)
        nc.vector.memset(z_sb, 0.0)
        xt_sb = singles.tile([128, KT, B], F16)
        if _VARIANT == "compute_only":
            k_fix = singles.tile([128, S16], F16)
            nc.vector.memset(k_fix, 0.01)
            k8_fix = singles.tile([128, max(S8, 128)], F8)
            nc.vector.memset(k8_fix, 0.01)
            v_fix = singles.tile([128, SV], F8)
            nc.vector.memset(v_fix, 0.01)

        def body():
            dma_only = _VARIANT == "dma_only"
            nc.sync.dma_start(out=xt_sb, in_=xt[:, :, :])

            # ---- QKV projections: out[b, f] accumulated over 32 k-tiles ----
            q_ps = pps.tile([B, FL], F32, tag="qp")
            k_ps = pps.tile([B, FL], F32, tag="kp")
            v_ps = pps.tile([B, FL], F32, tag="vp")
            for kc in range(KT // wq_chunk):
                w_sb = wpool.tile([128, wq_chunk, 2 * FL], F16, tag="w")
                nc.sync.dma_start(
                    out=w_sb, in_=wqkv[:, kc * wq_chunk : (kc + 1) * wq_chunk, :]
                )
                wv_sb = w8pool.tile([128, wq_chunk, FL], F8, tag="wv")
                nc.sync.dma_start(
                    out=wv_sb, in_=wv8[:, kc * wq_chunk : (kc + 1) * wq_chunk, :]
                )
                if dma_only:
                    continue
                for j in range(wq_chunk):
                    kt = kc * wq_chunk + j
                    lhs = xt_sb[:, kt, :]
                    st, sp = kt == 0, kt == KT - 1
                    nc.tensor.matmul(
                        q_ps, lhsT=lhs, rhs=w_sb[:, j, 0:FL], start=st, stop=sp
                    )
                    nc.tensor.matmul(
                        k_ps, lhsT=lhs, rhs=w_sb[:, j, FL : 2 * FL], start=st, stop=sp
                    )
                    nc.tensor.matmul(
                        v_ps, lhsT=lhs, rhs=wv_sb[:, j, :], start=st, stop=sp
                    )
            if not dma_only:
                nc.vector.tensor_copy(q_sb, q_ps)
                nc.vector.tensor_copy(k_sb, k_ps)
                nc.vector.tensor_copy(vn_sb, v_ps)

            # ---- score_last[b] = q . k_new per head; P_last = exp ----
            for h in range(HL if not dma_only else 0):
                sl_tmp = vscp.tile([B, DK], F32, tag="sl_tmp")
                sl_h = vscp.tile([B, 1], F32, tag="sl_h")
                nc.vector.tensor_mul(
                    sl_tmp,
                    q_sb[:, h * DK : (h + 1) * DK],
                    k_sb[:, h * DK : (h + 1) * DK],
                )
                nc.vector.reduce_sum(out=sl_h, in_=sl_tmp, axis=mybir.AxisListType.X)
                nc.scalar.activation(
                    out=pl_sb[:, h : h + 1], in_=sl_h, func=AF.Exp,
                    bias=nln32_sb,
                )

            # ---- transpose q, k_new into [d, b] column layout per head ----
            for h in range(HL if not dma_only else 0):
                qt_ps = mips.tile([DK, B], F32, tag="mi")
                nc.tensor.matmul(
                    qt_ps, lhsT=q_sb[:, h * DK : (h + 1) * DK], rhs=ident[:B, :B],
                    start=True, stop=True,
                )
                nc.vector.tensor_copy(qT_sb[:, h * B : (h + 1) * B], qt_ps)
                kt_ps = mips.tile([DK, B], F32, tag="mi")
                nc.tensor.matmul(
                    kt_ps, lhsT=k_sb[:, h * DK : (h + 1) * DK], rhs=ident[:B, :B],
                    start=True, stop=True,
                )
                nc.vector.tensor_copy(kTn_sb[:, h * B : (h + 1) * B], kt_ps)

            # hoist Wo weight loads ahead of the KV stream (trims the tail)
            wo_tiles = []
            if not dma_only:
                for hc in range(HL // wo_chunk):
                    wo_sb = wop.tile([128, wo_chunk, D], F16, tag="wo")
                    nc.sync.dma_start(
                        out=wo_sb,
                        in_=wo[:, hc * wo_chunk : (hc + 1) * wo_chunk, :],
                    )
                    wo_tiles.append(wo_sb)

            # ---- attention over the cache, head by head ----
            if dma_only:
                for h in range(HL):
                    for g in range(B // G):
                        k_sb_t = kpool.tile([128, G, S16], F16, tag="kv")
                        nc.sync.dma_start(
                            out=k_sb_t,
                            in_=k_d[h, g * G : (g + 1) * G].rearrange("g p f -> p g f"),
                        )
                        if NT8 > 0:
                            k8_sb_t = k8pool.tile([128, G, S8], F8, tag="k8")
                            nc.sync.dma_start(
                                out=k8_sb_t,
                                in_=k8_d[h, g * G : (g + 1) * G].rearrange(
                                    "g p f -> p g f"
                                ),
                            )
                        v_sb_t = vpool.tile([128, G, SV], F8, tag="vv")
                        nc.sync.dma_start(
                            out=v_sb_t,
                            in_=v_d[h, g * G : (g + 1) * G].rearrange("g p f -> p g f"),
                        )
                for hc in range(HL // wo_chunk):
                    wo_sb = wop.tile([128, wo_chunk, D], F16, tag="wo")
                    nc.sync.dma_start(
                        out=wo_sb, in_=wo[:, hc * wo_chunk : (hc + 1) * wo_chunk, :]
                    )
                nc.sync.dma_start(out=z_d[:, :], in_=z_sb)
                return
            for h in range(HL):
                ao_ps = aops.tile([DK, B], F32, tag="ao")
                # correction term: AO[d, b] += P_last[b] * v_new[b, d]
                # (transposed-by-identity matmul opens the accumulation group)
                vsc = vscp.tile([B, DK], F32, tag="vsc")
                nc.vector.tensor_scalar_mul(
                    vsc,
                    in0=vn_sb[:, h * DK : (h + 1) * DK],
                    scalar1=pl_sb[:, h : h + 1],
                )
                no_pv = NT == 1 and r_new == 0  # S_eff == 1: no cache matmuls
                nc.tensor.matmul(
                    ao_ps, lhsT=vsc, rhs=ident[:B, :B], start=True, stop=no_pv
                )

                pending = []  # software-pipeline PV one bh behind scores

                def emit_pv(ent, is_last_b):
                    b_, p_sb_, v_sb_ = ent
                    n_full = NT - 1
                    for t in range(n_full):
                        last = t == n_full - 1 and r_new == 0 and is_last_b
                        nc.tensor.matmul(
                            ao_ps[:, b_ : b_ + 1],
                            lhsT=v_sb_[:, t * DK : (t + 1) * DK],
                            rhs=p_sb_[:, t : t + 1],
                            start=False,
                            stop=last,
                        )
                    if r_new > 0:
                        nc.tensor.matmul(
                            ao_ps[:, b_ : b_ + 1],
                            lhsT=v_sb_[:r_new, (NT - 1) * DK : NT * DK],
                            rhs=p_sb_[:r_new, NT - 1 : NT],
                            start=False,
                            stop=is_last_b,
                        )

                for g in range(B // G):
                    if _VARIANT == "compute_only":
                        k_grp, k8_grp, v_grp = None, None, None
                    else:
                        k_grp = kpool.tile([128, G, S16], F16, tag="kv")
                        nc.sync.dma_start(
                            out=k_grp,
                            in_=k_d[h, g * G : (g + 1) * G].rearrange("g p f -> p g f"),
                        )
                        if NT8 > 0:
                            k8_grp = k8pool.tile([128, G, S8], F8, tag="k8")
                            nc.sync.dma_start(
                                out=k8_grp,
                                in_=k8_d[h, g * G : (g + 1) * G].rearrange(
                                    "g p f -> p g f"
                                ),
                            )
                        v_grp = vpool.tile([128, G, SV], F8, tag="vv")
                        nc.sync.dma_start(
                            out=v_grp,
                            in_=v_d[h, g * G : (g + 1) * G].rearrange("g p f -> p g f"),
                        )
                    for j in range(G):
                        b = g * G + j
                        col = h * B + b
                        k_sb_b = k_fix if k_grp is None else k_grp[:, j, :]
                        k8_sb_b = (
                            (k8_fix if k8_grp is None else k8_grp[:, j, :])
                            if NT8 > 0
                            else None
                        )
                        v_sb_b = v_fix if v_grp is None else v_grp[:, j, :]
                        if _VARIANT == "full":
                            # scatter k_new into the cache column for start_pos
                            if scatter_in_k8:
                                nc.vector.tensor_copy(
                                    out=k8_sb_b[:, S_eff - 1 - S16 : S_eff - S16],
                                    in_=kTn_sb[:, col : col + 1],
                                )
                            else:
                                nc.vector.tensor_copy(
                                    out=k_sb_b[:, S_eff - 1 : S_eff],
                                    in_=kTn_sb[:, col : col + 1],
                                )
                        sc_ps = scps.tile([128, NT], F32, tag="sc")
                        for t in range(NT):
                            if t < NT16:
                                lhs_t = k_sb_b[:, t * 128 : (t + 1) * 128]
                            else:
                                tt = t - NT16
                                lhs_t = k8_sb_b[:, tt * 128 : (tt + 1) * 128]
                            nc.tensor.matmul(
                                sc_ps[:, t : t + 1],
                                lhsT=lhs_t,
                                rhs=qT_sb[:, col : col + 1],
                                start=True,
                                stop=True,
                            )
                        p_sb = ppool.tile([128, NT], F16, tag="p")
                        nc.scalar.activation(
                            out=p_sb,
                            in_=sc_ps,
                            func=AF.Exp,
                            accum_out=zsum_sb[:, col : col + 1],
                        )
                        pending.append((b, p_sb, v_sb_b))
                        if len(pending) == 2:
                            emit_pv(pending.pop(0), is_last_b=False)
                emit_pv(pending.pop(0), is_last_b=True)

                nc.vector.tensor_copy(ao_sb[:, h * B : (h + 1) * B], ao_ps)
                # Z per (h, b): sum zsum over partitions via ones-matmul
                z_ps = mips.tile([1, B], F32, tag="mi")
                nc.tensor.matmul(
                    z_ps,
                    lhsT=ones_col,
                    rhs=zsum_sb[:, h * B : (h + 1) * B],
                    start=True,
                    stop=True,
                )
                nc.vector.tensor_copy(z_sb[:, h * B : (h + 1) * B], z_ps)

            # ---- per-head output projection (unnormalized) ----
            for hc in range(HL // wo_chunk):
                wo_sb = wo_tiles[hc]
                for j in range(wo_chunk):
                    h = hc * wo_chunk + j
                    y_sb = ysbp.tile([B, D], F16, tag="ysb")
                    for oc in range(D // 512):
                        y_ps = mips.tile([B, 512], F32, tag="mi")
                        nc.tensor.matmul(
                            y_ps,
                            lhsT=ao_sb[:, h * B : (h + 1) * B],
                            rhs=wo_sb[:, j, oc * 512 : (oc + 1) * 512],
                            start=True,
                            stop=True,
                        )
                        nc.vector.tensor_copy(y_sb[:, oc * 512 : (oc + 1) * 512], y_ps)
                    nc.sync.dma_start(out=y_d[h], in_=y_sb)

            nc.sync.dma_start(out=z_d[:, :], in_=z_sb)

        if repeat == 1:
            body()
        else:
            with tc.For_i(0, repeat, 1):
                body()

    nc.compile()
    return nc


def _prep_inputs(x, k_cache, v_cache, Wq, Wk, Wv, Wo, S_eff):
    """Host-side sharding + layout prep. Returns per-core input dicts."""
    NT = (S_eff + 127) // 128
    S_pad = NT * 128
    scale = np.float32(DK ** -0.5)

    x2 = np.asarray(x, dtype=np.float32).reshape(B, D)
    xt_tiled = np.ascontiguousarray(
        x2.T.reshape(KT, 128, B).transpose(1, 0, 2).astype(np.float16)
    )  # [128, KT, B]

    k_cache = np.asarray(k_cache, dtype=np.float32)
    v_cache = np.asarray(v_cache, dtype=np.float32)

    NT8 = min(K8_TILES, NT - 1)
    S8 = NT8 * 128
    S16 = S_pad - S8
    # K^T slabs per (h,b): leading positions [0, S16) fp16, trailing as e3m4
    k_all = np.ascontiguousarray(
        k_cache[:, :, :S16, :].transpose(1, 0, 3, 2)
    ).astype(np.float16)
    k8_all = np.zeros((H, B, 128, S8), dtype=ml_dtypes.float8_e3m4)
    k8_all[:, :, :, : S_eff - S16] = (
        k_cache[:, :, S16:S_eff, :].transpose(1, 0, 3, 2)
    ).astype(ml_dtypes.float8_e3m4)
    # V-tiled slab per (h,b): [128, NT*DK] e3m4
    v_src = np.zeros((H, B, S_pad, DK), dtype=np.float32)
    v_src[:, :, :S_eff] = v_cache[:, :, :S_eff].transpose(1, 0, 2, 3)
    v_all = np.ascontiguousarray(
        v_src.reshape(H, B, NT, 128, DK)
        .transpose(0, 1, 3, 2, 4)
        .reshape(H, B, 128, NT * DK)
    ).astype(ml_dtypes.float8_e3m4)
    del v_src

    Wq = np.asarray(Wq, dtype=np.float32)
    Wk = np.asarray(Wk, dtype=np.float32)
    Wv = np.asarray(Wv, dtype=np.float32)
    Wo = np.asarray(Wo, dtype=np.float32)

    in_maps = []
    for c in range(NCORES):
        rows = slice(c * FL, (c + 1) * FL)
        wqkv_c = np.concatenate(
            [Wq[rows].T * scale, Wk[rows].T], axis=1
        )  # (D, 2*FL)
        wqkv_tiled = np.ascontiguousarray(
            wqkv_c.reshape(KT, 128, 2 * FL).transpose(1, 0, 2).astype(np.float16)
        )
        wv8_tiled = np.ascontiguousarray(
            (Wv[rows].T * np.float32(32.0))
            .reshape(KT, 128, FL).transpose(1, 0, 2)
        ).astype(ml_dtypes.float8_e3m4)
        wo_c = Wo[:, rows].T  # (FL, D)
        wo_tiled = np.ascontiguousarray(
            wo_c.reshape(HL, 128, D).transpose(1, 0, 2).astype(np.float16)
        )
        in_maps.append(
            {
                "xt": xt_tiled,
                "wqkv": wqkv_tiled,
                "wv8": wv8_tiled,
                "wo": wo_tiled,
                "kc": np.ascontiguousarray(k_all[c * HL : (c + 1) * HL]),
                "k8": np.ascontiguousarray(k8_all[c * HL : (c + 1) * HL]),
                "vc": np.ascontiguousarray(v_all[c * HL : (c + 1) * HL]),
            }
        )
    return in_maps


def _combine(results, S_eff):
    """Host-side unshard: divide per-head partials by Z, sum everything."""
    NT = (S_eff + 127) // 128
    n_pad = NT * 128 - S_eff
    y = np.zeros((B, D), dtype=np.float64)
    for c in range(NCORES):
        z = results[c]["z"].reshape(HL, B).astype(np.float64) - n_pad
        yp = results[c]["y"].astype(np.float64)  # (HL, B, D)
        y += (yp / z[:, :, None]).sum(axis=0)
    return y.astype(np.float32).reshape(B, 1, D)


def kernel(x, k_cache, v_cache, Wq, Wk, Wv, Wo, start_pos):
    start_pos = int(np.asarray(start_pos))
    S_eff = start_pos + 1
    in_maps = _prep_inputs(x, k_cache, v_cache, Wq, Wk, Wv, Wo, S_eff)
    nc = _PROGRAM_CACHE.get(S_eff)
    if nc is None:
        nc = build_program(S_eff)
        _PROGRAM_CACHE[S_eff] = nc
    res = run_bass_kernel_spmd(nc, in_maps, core_ids=list(range(NCORES)))
    return _combine(res.results, S_eff)
